# revision 1
# baseline (speedup 1.0000x reference)
"""MFA per-component log-likelihood kernel for 8x TRN2 NeuronCores.

Math: reference computes, for K=128 mixture components with Woodbury
factor structure Sigma_k = D_k^2 + A_k A_k^T (via the small l=16 matrix
L = I + A^T iD A):

  out[n,k] = PI[k] - 0.5*(d*log2pi + logdetSigma[k] + m_d[n,k])
  m_d = term1 - quad,  term1 = sum_d iD (x-MU)^2,  quad = y^T iL y

Host-side (tiny, O(K*d*l)): compute iL = C C^T (Cholesky), fold
everything into three weight matrices so the device does only:

  out[n,k] = base[k] + sum_m (x_n @ Gw[:,k*16+m])^2 + x_n @ Wx[:,k]
             + (x_n^2) @ Wxx[:,k]

where (with G'_k = iD A C / sqrt(2), h'_k = (MU^T iD A C)/sqrt(2)):
  Gw[:, k*16+m] = G'_k[:, m]
  Wx[:, k]      = (iD*MU)_k - 2 * G'_k @ h'_k
  Wxx[:, k]     = -0.5 * iD_k
  base[k]       = PI[k] - 0.5*(d*log2pi + logdetSigma_k + sum_d iD MU^2)
                  + sum_m h'^2

Device: per 128-row tile of x (x pre-transposed on host so the
contraction dim d sits on SBUF partitions), 8 fp32 PE matmuls per
512-col block accumulate in PSUM; ScalarE squares PSUM->SBUF; VectorE
does the group-of-16 reduce and the final adds.  Sharding: rows N=16384
split across 8 cores (2048 rows each); params replicated.
"""

import math

import numpy as np

K, D_FEAT, L_FAC, N = 128, 1024, 16, 16384
N_CORES = 8
N_SHARD = N // N_CORES            # 2048 rows per core
NT = N_SHARD // 128               # 16 row tiles per core
DT = D_FEAT // 128                # 8 contraction tiles
GCOLS = K * L_FAC                 # 2048 factor columns
WCOLS = GCOLS + K                 # 2176 = [Gw | Wx]

_CACHE = {}


def _get_nc():
    if "nc" in _CACHE:
        return _CACHE["nc"]

    import concourse.bass as bass
    import concourse.tile as tile
    from concourse import bacc, mybir

    f32 = mybir.dt.float32
    f32r = mybir.dt.float32r
    nc = bacc.Bacc("TRN2", target_bir_lowering=False, debug=False,
                   num_devices=N_CORES)

    xT = nc.dram_tensor("xT", [D_FEAT, N_SHARD], f32, kind="ExternalInput").ap()
    wmov = nc.dram_tensor("wmov", [128, DT, WCOLS], f32, kind="ExternalInput").ap()
    wxx = nc.dram_tensor("wxx", [128, DT, K], f32, kind="ExternalInput").ap()
    baser = nc.dram_tensor("baser", [128, K], f32, kind="ExternalInput").ap()
    out = nc.dram_tensor("out", [N_SHARD, K], f32, kind="ExternalOutput").ap()

    with tile.TileContext(nc) as tc:
        with (
            tc.tile_pool(name="singles", bufs=1) as singles,
            tc.tile_pool(name="xtp", bufs=4) as xtp,
            tc.tile_pool(name="xsqp", bufs=4) as xsqp,
            tc.tile_pool(name="sqp", bufs=8) as sqp,
            tc.tile_pool(name="ep", bufs=3) as ep,
            tc.tile_pool(name="gps", bufs=4, space="PSUM") as gps,
            tc.tile_pool(name="tpsa", bufs=2, space="PSUM") as tpsa,
            tc.tile_pool(name="tpsb", bufs=2, space="PSUM") as tpsb,
        ):
            wm = singles.tile([128, DT, WCOLS], f32r, tag="wm")
            nc.sync.dma_start(out=wm, in_=wmov.bitcast(f32r))
            wx = singles.tile([128, DT, K], f32, tag="wx")
            nc.sync.dma_start(out=wx, in_=wxx)
            bs = singles.tile([128, K], f32, tag="bs")
            nc.sync.dma_start(out=bs, in_=baser)

            # The LDWEIGHTS instruction can carry only one semaphore wait,
            # so a matmul may depend on at most one not-yet-observed
            # processor.  Touch each weight tensor with a throwaway matmul
            # first so the real matmuls only ever wait on their xt DMA.
            warm = gps.tile([128, 512], f32, tag="g")
            nc.tensor.matmul(warm, wm[:, 0, 0:128], wm[:, 0, 0:512],
                             start=True, stop=True)
            warm2 = gps.tile([128, 512], f32, tag="g")
            nc.tensor.matmul(warm2[:, 0:128], wx[:, 0, 0:128], wx[:, 0, 0:128],
                             start=True, stop=True)
            # same trick for VectorE's first read of bs
            warm3 = ep.tile([128, 1], f32, tag="w3")
            nc.vector.tensor_copy(warm3, bs[:, 0:1])

            for i in range(NT):
                xt = xtp.tile([128, DT, 128], f32r, tag="xt")
                nc.sync.dma_start(
                    out=xt,
                    in_=xT[:, i * 128:(i + 1) * 128].rearrange(
                        "(j p) n -> p j n", p=128).bitcast(f32r),
                )
                xsq = xsqp.tile([128, DT, 128], f32, tag="xsq")
                nc.scalar.square(xsq, xt)

                quad = ep.tile([128, K], f32, tag="quad")
                for cb in range(4):
                    ps = gps.tile([128, 512], f32, tag="g")
                    for j in range(DT):
                        nc.tensor.matmul(
                            ps, xt[:, j, :],
                            wm[:, j, cb * 512:(cb + 1) * 512],
                            start=(j == 0), stop=(j == DT - 1))
                    sq = sqp.tile([128, 512], f32, tag="sq")
                    nc.scalar.square(sq, ps)
                    nc.vector.reduce_sum(
                        out=quad[:, cb * 32:(cb + 1) * 32],
                        in_=sq.rearrange("p (g i) -> p g i", i=L_FAC),
                        axis=mybir.AxisListType.X,
                    )

                psa = tpsa.tile([128, K], f32, tag="ta")
                for j in range(DT):
                    nc.tensor.matmul(psa, xt[:, j, :],
                                     wm[:, j, GCOLS:],
                                     start=(j == 0), stop=(j == DT - 1))
                psb = tpsb.tile([128, K], f32, tag="tb")
                for j in range(DT):
                    nc.tensor.matmul(psb, xsq[:, j, :],
                                     wx[:, j, :],
                                     start=(j == 0), stop=(j == DT - 1))

                u = ep.tile([128, K], f32, tag="u")
                nc.vector.tensor_add(out=u, in0=quad, in1=bs)
                nc.vector.tensor_add(out=u, in0=u, in1=psa)
                nc.vector.tensor_add(out=u, in0=u, in1=psb)
                nc.gpsimd.dma_start(out=out[i * 128:(i + 1) * 128, :], in_=u)

    nc.finalize()
    _CACHE["nc"] = nc
    return nc


def _host_params(PI, MU, A, D):
    PI64 = PI.astype(np.float64)
    MU64 = MU.astype(np.float64)
    A64 = A.astype(np.float64)
    D64 = D.astype(np.float64)

    iD = D64 ** -2.0                                   # (K, d)
    iDA = iD[:, :, None] * A64                         # (K, d, l)
    Lm = np.eye(L_FAC)[None] + np.einsum("kdl,kdm->klm", A64, iDA)
    iL = np.linalg.inv(Lm)
    C = np.linalg.cholesky(iL)                         # iL = C C^T
    s = 1.0 / math.sqrt(2.0)
    G = np.einsum("kdl,klm->kdm", iDA, C) * s          # (K, d, l)
    b = np.einsum("kd,kdl->kl", MU64, iDA)             # (K, l)
    h = np.einsum("kl,klm->km", b, C) * s              # (K, l)

    Gw = G.transpose(1, 0, 2).reshape(D_FEAT, GCOLS)   # col k*16+m
    Wx = (iD * MU64).T - 2.0 * np.einsum("kdm,km->kd", G, h).T
    Wxx = -0.5 * iD.T

    det_L = np.linalg.slogdet(Lm)[1]
    log_det_sigma = det_L - np.sum(np.log(iD), axis=1)
    c1 = np.sum(iD * MU64 * MU64, axis=1)
    hsq = np.sum(h * h, axis=1)
    base = PI64 - 0.5 * (D_FEAT * math.log(2.0 * math.pi)
                         + log_det_sigma + c1) + hsq

    wcat = np.concatenate([Gw, Wx], axis=1)            # (d, 2176)
    wmov = np.ascontiguousarray(
        wcat.reshape(DT, 128, WCOLS).transpose(1, 0, 2)).astype(np.float32)
    wxx = np.ascontiguousarray(
        Wxx.reshape(DT, 128, K).transpose(1, 0, 2)).astype(np.float32)
    baser = np.broadcast_to(base.astype(np.float32), (128, K)).copy()
    return wmov, wxx, baser


def kernel(x, PI, MU, A, D, _trace=False):
    from concourse.bass_utils import run_bass_kernel_spmd

    x = np.asarray(x, dtype=np.float32)
    wmov, wxx, baser = _host_params(
        np.asarray(PI), np.asarray(MU), np.asarray(A), np.asarray(D))

    in_maps = []
    for c in range(N_CORES):
        xs = x[c * N_SHARD:(c + 1) * N_SHARD]
        in_maps.append({
            "xT": np.ascontiguousarray(xs.T),
            "wmov": wmov,
            "wxx": wxx,
            "baser": baser,
        })

    nc = _get_nc()
    res = run_bass_kernel_spmd(nc, in_maps, list(range(N_CORES)),
                               trace=_trace)
    _CACHE["last_results"] = res
    return np.concatenate([res.results[c]["out"] for c in range(N_CORES)],
                          axis=0)



# revision 3
# speedup vs baseline: 2.0071x; 2.0071x over previous
"""MFA per-component log-likelihood kernel for 8x TRN2 NeuronCores.

Math: out[n,k] = base[k] + sum_m (x_n . g_km)^2 + x_n . Wx_k + (x_n^2) . Wxx_k
with g/Wx/base from the Woodbury factorization (host-side, tiny).

Device strategy (per core, N_SHARD=2048 rows, output TRANSPOSED [K, n]):
  - Weights-stationary fp8 (e4m3) DoubleRow matmuls for the factor ("quad")
    columns: stationary = Gw block [128d, 2, 128cols], moving = x_fp8
    [128d, 2, 512n] -> psum y-block [128cols, 512n] at 0.5 cyc/row.
  - ScalarE/DVE square psum with scale 1/64 into fp8 "sq" pairs.
  - Group-of-16 reduction on the PE: fp8 DoubleRow matmul against a
    constant 0/4 block-indicator matrix (S2), accumulating quad directly
    into the per-n-block accumulator psum bank [128k, 512n].
  - Linear term x.Wx: fp8 DoubleRow, stationary Wx block, same moving x,
    accumulated into the same acc bank.
  - x^2 . Wxx + base: fp16 matmuls (host-computed xsq fp16), 9th chunk
    carries a ones-row/base-row pair that adds base[k] exactly.
  - DVE copies acc psum -> SBUF, DMA out. Host transposes [K,N] -> [N,K].

Sharding: rows N=16384 split across 8 cores; params replicated.
"""

import math

import numpy as np

K, D_FEAT, L_FAC, N = 128, 1024, 16, 16384
N_CORES = 8
N_SHARD = N // N_CORES            # 2048 rows per core
NB = N_SHARD // 512               # 4 moving blocks of 512 rows
J2 = 4                            # DoubleRow contraction chunks (256 each)
JJ = 9                            # psb fp16 chunks (8 data + 1 base row)
CB = 16                           # 128-col blocks of factor columns
NPAIR = CB // 2                   # S2 pair matmuls per n-block
GCOLS = K * L_FAC                 # 2048 factor columns
WCOLS = GCOLS + K                 # 2176 = [Gw | Wx]
SG = 32.0                         # Gw fp8 scale
SQ_SCALE = 1.0 / 64.0             # scalar: sq = (psum/64)^2 = y^2/4
DVE_SQ_SCALE = 1.0 / 4096.0       # dve: sq = psum*(psum/4096) = y^2/4
S2_VAL = 4.0                      # un-scales sq in the group-sum matmul

_CACHE = {}


def _get_nc():
    if "nc" in _CACHE:
        return _CACHE["nc"]

    import concourse.bass as bass
    import concourse.tile as tile
    from concourse import bacc, mybir

    f32 = mybir.dt.float32
    f16 = mybir.dt.float16
    f8 = mybir.dt.float8e4
    DR = mybir.MatmulPerfMode.DoubleRow
    nc = bacc.Bacc("TRN2", target_bir_lowering=False, debug=False,
                   num_devices=N_CORES)

    xq = nc.dram_tensor("xq", [128, J2, 2, N_SHARD], f8, kind="ExternalInput").ap()
    xsqh = nc.dram_tensor("xsqh", [128, JJ, N_SHARD], f16, kind="ExternalInput").ap()
    wq = nc.dram_tensor("wq", [128, J2, 2, WCOLS], f8, kind="ExternalInput").ap()
    wxxh = nc.dram_tensor("wxxh", [128, JJ, K], f16, kind="ExternalInput").ap()
    s2 = nc.dram_tensor("s2", [128, NPAIR, 2, K], f8, kind="ExternalInput").ap()
    outT = nc.dram_tensor("outT", [128, N_SHARD], f32, kind="ExternalOutput").ap()

    # wq column split points for the input DMA (so the PE can start early)
    WPIECES = [(0, 512), (512, 1024), (1024, 1536), (1536, WCOLS)]

    with tile.TileContext(nc) as tc:
        with (
            tc.tile_pool(name="singles", bufs=1) as singles,
            tc.tile_pool(name="sqpool", bufs=3) as sqpool,
            tc.tile_pool(name="upool", bufs=2) as upool,
            tc.tile_pool(name="qp", bufs=4, space="PSUM") as qp,
            tc.tile_pool(name="accp", bufs=4, space="PSUM") as accp,
        ):
            wq_s = singles.tile([128, J2, 2, WCOLS], f8, tag="wq")
            xq_s = singles.tile([128, J2, 2, N_SHARD], f8, tag="xq")
            xs_s = singles.tile([128, JJ, N_SHARD], f16, tag="xs")
            wxx_s = singles.tile([128, JJ, K], f16, tag="wxx")
            s2_s = singles.tile([128, NPAIR, 2, K], f8, tag="s2")

            # DMA issue order is the prefetch schedule: earliest-needed first.
            nc.sync.dma_start(out=wq_s[:, :, :, 0:512], in_=wq[:, :, :, 0:512])
            nc.sync.dma_start(out=xq_s[:, :, :, 0:512], in_=xq[:, :, :, 0:512])
            for lo, hi in WPIECES[1:]:
                nc.sync.dma_start(out=wq_s[:, :, :, lo:hi], in_=wq[:, :, :, lo:hi])
            nc.sync.dma_start(out=s2_s, in_=s2)
            nc.sync.dma_start(out=wxx_s, in_=wxxh)
            for nb in range(1, NB):
                nc.sync.dma_start(out=xq_s[:, :, :, nb * 512:(nb + 1) * 512],
                                  in_=xq[:, :, :, nb * 512:(nb + 1) * 512])
            for nb in range(NB):
                nc.sync.dma_start(out=xs_s[:, :, nb * 512:(nb + 1) * 512],
                                  in_=xsqh[:, :, nb * 512:(nb + 1) * 512])

            # A matmul (its LDWEIGHTS) can wait on at most one not-yet-observed
            # semaphore, so touch each weight tensor with a throwaway matmul
            # first; real matmuls then wait only on their data-side producer.
            for j2 in range(J2):
                warm = qp.tile([128, 512], f32, tag="q")
                nc.tensor.matmul(warm, wq_s[:, j2, :, 0:128],
                                 wq_s[:, j2, :, 0:512],
                                 start=True, stop=True, perf_mode=DR)
            warm2 = qp.tile([128, 512], f32, tag="q")
            nc.tensor.matmul(warm2[:, 0:128], s2_s[:, 0], s2_s[:, 0],
                             start=True, stop=True, perf_mode=DR)
            warm3 = qp.tile([128, 512], f32, tag="q")
            nc.tensor.matmul(warm3[:, 0:128], wxx_s[:, 0], wxx_s[:, 0],
                             start=True, stop=True)

            accs = []
            psb_done = 0

            def emit_psb(nb):
                acc = accs[nb]
                nbs = slice(nb * 512, (nb + 1) * 512)
                for jj in range(JJ):
                    nc.tensor.matmul(acc, wxx_s[:, jj], xs_s[:, jj, nbs],
                                     start=False, stop=(jj == JJ - 1))
                u = upool.tile([128, 512], f32, tag="u")
                nc.vector.tensor_copy(u, acc)
                nc.gpsimd.dma_start(out=outT[:, nbs], in_=u)

            for nb in range(NB):
                acc = accp.tile([128, 512], f32, tag="acc")
                accs.append(acc)
                nbs = slice(nb * 512, (nb + 1) * 512)
                pending = None
                for pair in range(NPAIR):
                    sq_t = sqpool.tile([128, 2, 512], f8, tag="sq")
                    for r in range(2):
                        cb = 2 * pair + r
                        q = qp.tile([128, 512], f32, tag="q")
                        for j2 in range(J2):
                            nc.tensor.matmul(
                                q, wq_s[:, j2, :, cb * 128:(cb + 1) * 128],
                                xq_s[:, j2, :, nbs],
                                start=(j2 == 0), stop=(j2 == J2 - 1),
                                perf_mode=DR)
                        nc.scalar.activation(
                            sq_t[:, r, :], q,
                            mybir.ActivationFunctionType.Square,
                            scale=SQ_SCALE)
                    # defer the group-sum one pair so the square can finish
                    if pending is not None:
                        p_pair, p_sq = pending
                        nc.tensor.matmul(acc, s2_s[:, p_pair], p_sq,
                                         start=(p_pair == 0), stop=False,
                                         perf_mode=DR)
                    pending = (pair, sq_t)
                p_pair, p_sq = pending
                nc.tensor.matmul(acc, s2_s[:, p_pair], p_sq,
                                 start=False, stop=False, perf_mode=DR)
                # linear term x . Wx (fp8, same moving x)
                for j2 in range(J2):
                    nc.tensor.matmul(acc, wq_s[:, j2, :, GCOLS:WCOLS],
                                     xq_s[:, j2, :, nbs],
                                     start=False, stop=False, perf_mode=DR)
                # deferred psb sections (waits until xsq DMA surely landed)
                if nb >= 1 and psb_done < nb - 0:
                    emit_psb(psb_done)
                    psb_done += 1

            while psb_done < NB:
                emit_psb(psb_done)
                psb_done += 1

    nc.finalize()
    _CACHE["nc"] = nc
    return nc


def _host_params(PI, MU, A, D):
    import ml_dtypes
    FP8 = ml_dtypes.float8_e4m3

    PI64 = PI.astype(np.float64)
    MU64 = MU.astype(np.float64)
    A64 = A.astype(np.float64)
    D64 = D.astype(np.float64)

    iD = D64 ** -2.0                                   # (K, d)
    iDA = iD[:, :, None] * A64                         # (K, d, l)
    Lm = np.eye(L_FAC)[None] + np.einsum("kdl,kdm->klm", A64, iDA)
    iL = np.linalg.inv(Lm)
    C = np.linalg.cholesky(iL)                         # iL = C C^T
    s = 1.0 / math.sqrt(2.0)
    G = np.einsum("kdl,klm->kdm", iDA, C) * s          # (K, d, l)
    b = np.einsum("kd,kdl->kl", MU64, iDA)             # (K, l)
    h = np.einsum("kl,klm->km", b, C) * s              # (K, l)

    Gw = G.transpose(1, 0, 2).reshape(D_FEAT, GCOLS)   # col k*16+m
    Wx = (iD * MU64).T - 2.0 * np.einsum("kdm,km->kd", G, h).T
    Wxx = -0.5 * iD.T                                  # (d, K)

    det_L = np.linalg.slogdet(Lm)[1]
    log_det_sigma = det_L - np.sum(np.log(iD), axis=1)
    c1 = np.sum(iD * MU64 * MU64, axis=1)
    hsq = np.sum(h * h, axis=1)
    base = PI64 - 0.5 * (D_FEAT * math.log(2.0 * math.pi)
                         + log_det_sigma + c1) + hsq

    wcat = np.concatenate([Gw * SG, Wx], axis=1).astype(np.float32)  # (d, 2176)
    wq = np.ascontiguousarray(
        wcat.astype(FP8).reshape(J2, 2, 128, WCOLS).transpose(2, 0, 1, 3))

    wxxh = np.zeros((128, JJ, K), dtype=np.float16)
    wxxh[:, 0:8, :] = Wxx.astype(np.float32).reshape(8, 128, K).transpose(1, 0, 2)
    wxxh[0, 8, :] = base.astype(np.float16)

    s2 = np.zeros((128, NPAIR, 2, K), dtype=np.float32)
    p_idx = np.arange(128)
    for pair in range(NPAIR):
        for r in range(2):
            cb = 2 * pair + r
            s2[p_idx, pair, r, cb * 8 + p_idx // 16] = S2_VAL
    s2 = s2.astype(FP8)

    return wq, wxxh, s2


def kernel(x, PI, MU, A, D, _trace=False):
    from concourse.bass_utils import run_bass_kernel_spmd
    import ml_dtypes
    FP8 = ml_dtypes.float8_e4m3

    x = np.asarray(x, dtype=np.float32)
    wq, wxxh, s2 = _host_params(
        np.asarray(PI), np.asarray(MU), np.asarray(A), np.asarray(D))

    in_maps = []
    for c in range(N_CORES):
        xs = x[c * N_SHARD:(c + 1) * N_SHARD]          # (2048, 1024)
        x8t = xs.astype(FP8).T                         # (1024, 2048)
        xq = np.ascontiguousarray(
            x8t.reshape(J2, 2, 128, N_SHARD).transpose(2, 0, 1, 3))
        xsqt = (xs * xs).astype(np.float16).T          # (1024, 2048)
        xsqh = np.empty((128, JJ, N_SHARD), dtype=np.float16)
        xsqh[:, 0:8, :] = xsqt.reshape(8, 128, N_SHARD).transpose(1, 0, 2)
        xsqh[:, 8, :] = 0.0
        xsqh[0, 8, :] = 1.0
        in_maps.append({
            "xq": xq,
            "xsqh": xsqh,
            "wq": wq,
            "wxxh": wxxh,
            "s2": s2,
        })

    nc = _get_nc()
    res = run_bass_kernel_spmd(nc, in_maps, list(range(N_CORES)),
                               trace=_trace)
    _CACHE["last_results"] = res
    outT = np.concatenate([res.results[c]["outT"] for c in range(N_CORES)],
                          axis=1)                      # (128, 16384)
    return np.ascontiguousarray(outT.T).astype(np.float32)


# revision 4
# speedup vs baseline: 2.1375x; 1.0650x over previous
"""MFA per-component log-likelihood kernel for 8x TRN2 NeuronCores.

Math: out[n,k] = base[k] + sum_m (x_n . g_km)^2 + x_n . Wx_k + (x_n^2) . Wxx_k
with g/Wx/base from the Woodbury factorization (host-side, tiny).

Device strategy (per core, N_SHARD=2048 rows, output TRANSPOSED [K, n]):
  - Weights-stationary fp8 (e4m3) DoubleRow matmuls for the factor ("quad")
    columns: stationary = Gw block [128d, 2, 128cols], moving = x_fp8
    [128d, 2, 512n] -> psum y-block [128cols, 512n], 256-deep contraction
    per streamed column (2x fp32 MAC rate).
  - ScalarE squares psum with scale 1/64 into fp8 "sq" pairs.
  - Group-of-16 reduction on the PE: fp8 DoubleRow matmul against a
    constant 0/4 block-indicator matrix (S2), accumulating quad directly
    into the per-n-block accumulator psum bank [128k, 512n].
  - Linear term x.Wx: fp8 DoubleRow, stationary Wx block, same moving x,
    accumulated into the same acc bank.
  - x^2 . Wxx: fp16 matmuls (host-computed xsq fp16) into the same bank.
  - DVE adds base (per-partition scalar) while copying acc psum -> SBUF,
    DMA out.  Host transposes [K,N] -> [N,K].

Sharding: rows N=16384 split across 8 cores; params replicated.
"""

import math

import numpy as np

K, D_FEAT, L_FAC, N = 128, 1024, 16, 16384
N_CORES = 8
N_SHARD = N // N_CORES            # 2048 rows per core
NB = N_SHARD // 512               # 4 moving blocks of 512 rows
J2 = 4                            # DoubleRow contraction chunks (256 each)
JJ = 8                            # psb fp16 chunks
CB = 16                           # 128-col blocks of factor columns
NPAIR = CB // 2                   # S2 pair matmuls per n-block
GCOLS = K * L_FAC                 # 2048 factor columns
WCOLS = GCOLS + K                 # 2176 = [Gw | Wx]
SG = 32.0                         # Gw fp8 scale
SQ_SCALE = 1.0 / 64.0             # scalar: sq = (psum/64)^2 = y^2/4
S2_VAL = 4.0                      # un-scales sq in the group-sum matmul

_CACHE = {}


def _get_nc():
    if "nc" in _CACHE:
        return _CACHE["nc"]

    import concourse.bass as bass
    import concourse.tile as tile
    from concourse import bacc, mybir

    f32 = mybir.dt.float32
    f16 = mybir.dt.float16
    f8 = mybir.dt.float8e4
    DR = mybir.MatmulPerfMode.DoubleRow
    nc = bacc.Bacc("TRN2", target_bir_lowering=False, debug=False,
                   num_devices=N_CORES)

    xq = nc.dram_tensor("xq", [128, J2, 2, N_SHARD], f8, kind="ExternalInput").ap()
    xsqh = nc.dram_tensor("xsqh", [128, JJ, N_SHARD], f16, kind="ExternalInput").ap()
    wq = nc.dram_tensor("wq", [128, J2, 2, WCOLS], f8, kind="ExternalInput").ap()
    wxxh = nc.dram_tensor("wxxh", [128, JJ, K], f16, kind="ExternalInput").ap()
    s2 = nc.dram_tensor("s2", [128, NPAIR, 2, K], f8, kind="ExternalInput").ap()
    bs = nc.dram_tensor("bs", [128, 1], f32, kind="ExternalInput").ap()
    outT = nc.dram_tensor("outT", [128, N_SHARD], f32, kind="ExternalOutput").ap()

    WPIECES = [(0, 512), (512, 1024), (1024, 1536), (1536, WCOLS)]

    with tile.TileContext(nc) as tc:
        with (
            tc.tile_pool(name="singles", bufs=1) as singles,
            tc.tile_pool(name="sqpool", bufs=3) as sqpool,
            tc.tile_pool(name="upool", bufs=2) as upool,
            tc.tile_pool(name="qp", bufs=4, space="PSUM") as qp,
            tc.tile_pool(name="accp", bufs=4, space="PSUM") as accp,
        ):
            wq_s = singles.tile([128, J2, 2, WCOLS], f8, tag="wq")
            xq_s = singles.tile([128, J2, 2, N_SHARD], f8, tag="xq")
            xs_s = singles.tile([128, JJ, N_SHARD], f16, tag="xs")
            wxx_s = singles.tile([128, JJ, K], f16, tag="wxx")
            s2_s = singles.tile([128, NPAIR, 2, K], f8, tag="s2")
            bs_s = singles.tile([128, 1], f32, tag="bs")

            # DMA issue order is the prefetch schedule: earliest-needed first.
            nc.sync.dma_start(out=wq_s[:, :, :, 0:512], in_=wq[:, :, :, 0:512])
            nc.sync.dma_start(out=xq_s[:, :, :, 0:512], in_=xq[:, :, :, 0:512])
            nc.sync.dma_start(out=s2_s, in_=s2)
            nc.sync.dma_start(out=wxx_s, in_=wxxh)
            nc.sync.dma_start(out=bs_s, in_=bs)
            for lo, hi in WPIECES[1:]:
                nc.sync.dma_start(out=wq_s[:, :, :, lo:hi], in_=wq[:, :, :, lo:hi])
            for nb in range(1, NB):
                nc.sync.dma_start(out=xq_s[:, :, :, nb * 512:(nb + 1) * 512],
                                  in_=xq[:, :, :, nb * 512:(nb + 1) * 512])
            for nb in range(NB):
                nc.sync.dma_start(out=xs_s[:, :, nb * 512:(nb + 1) * 512],
                                  in_=xsqh[:, :, nb * 512:(nb + 1) * 512])

            # A matmul (its LDWEIGHTS) can wait on at most one not-yet-observed
            # semaphore, so touch each weight tensor with a throwaway matmul
            # before its first real use; real matmuls then wait only on their
            # data-side producer.
            for j2 in range(J2):
                warm = qp.tile([128, 512], f32, tag="q")
                nc.tensor.matmul(warm, wq_s[:, j2, :, 0:128],
                                 wq_s[:, j2, :, 0:512],
                                 start=True, stop=True, perf_mode=DR)

            accs = []
            psb_done = 0

            def emit_psb(nb):
                acc = accs[nb]
                nbs = slice(nb * 512, (nb + 1) * 512)
                for jj in range(JJ):
                    nc.tensor.matmul(acc, wxx_s[:, jj], xs_s[:, jj, nbs],
                                     start=False, stop=(jj == JJ - 1))
                u = upool.tile([128, 512], f32, tag="u")
                nc.vector.tensor_scalar_add(out=u, in0=acc, scalar1=bs_s)
                nc.gpsimd.dma_start(out=outT[:, nbs], in_=u)

            for nb in range(NB):
                acc = accp.tile([128, 512], f32, tag="acc")
                accs.append(acc)
                nbs = slice(nb * 512, (nb + 1) * 512)
                pending = None
                for pair in range(NPAIR):
                    sq_t = sqpool.tile([128, 2, 512], f8, tag="sq")
                    for r in range(2):
                        cb = 2 * pair + r
                        q = qp.tile([128, 512], f32, tag="q")
                        for j2 in range(J2):
                            nc.tensor.matmul(
                                q, wq_s[:, j2, :, cb * 128:(cb + 1) * 128],
                                xq_s[:, j2, :, nbs],
                                start=(j2 == 0), stop=(j2 == J2 - 1),
                                perf_mode=DR)
                        nc.scalar.activation(
                            sq_t[:, r, :], q,
                            mybir.ActivationFunctionType.Square,
                            scale=SQ_SCALE)
                    if nb == 0 and pair == 0:
                        # warm s2 / wxxh before their first real matmuls
                        warm2 = qp.tile([128, 512], f32, tag="q")
                        nc.tensor.matmul(warm2[:, 0:128], s2_s[:, 0], s2_s[:, 0],
                                         start=True, stop=True, perf_mode=DR)
                        warm3 = qp.tile([128, 512], f32, tag="q")
                        nc.tensor.matmul(warm3[:, 0:128], wxx_s[:, 0],
                                         wxx_s[:, 0], start=True, stop=True)
                    # defer the group-sum one pair so the square can finish
                    if pending is not None:
                        p_pair, p_sq = pending
                        nc.tensor.matmul(acc, s2_s[:, p_pair], p_sq,
                                         start=(p_pair == 0), stop=False,
                                         perf_mode=DR)
                    pending = (pair, sq_t)
                    # spread deferred psb sections out of the critical tail
                    if nb == NB - 1 and pair == 4 and psb_done < 3:
                        emit_psb(psb_done)
                        psb_done += 1
                p_pair, p_sq = pending
                nc.tensor.matmul(acc, s2_s[:, p_pair], p_sq,
                                 start=False, stop=False, perf_mode=DR)
                # linear term x . Wx (fp8, same moving x)
                for j2 in range(J2):
                    nc.tensor.matmul(acc, wq_s[:, j2, :, GCOLS:WCOLS],
                                     xq_s[:, j2, :, nbs],
                                     start=False, stop=False, perf_mode=DR)
                # deferred psb sections (wait until xsq DMA surely landed)
                if 1 <= nb < NB - 1 and psb_done < nb:
                    emit_psb(psb_done)
                    psb_done += 1

            while psb_done < NB:
                emit_psb(psb_done)
                psb_done += 1

    nc.finalize()
    _CACHE["nc"] = nc
    return nc


def _host_params(PI, MU, A, D):
    import ml_dtypes
    FP8 = ml_dtypes.float8_e4m3

    PI64 = PI.astype(np.float64)
    MU64 = MU.astype(np.float64)
    A64 = A.astype(np.float64)
    D64 = D.astype(np.float64)

    iD = D64 ** -2.0                                   # (K, d)
    iDA = iD[:, :, None] * A64                         # (K, d, l)
    Lm = np.eye(L_FAC)[None] + np.einsum("kdl,kdm->klm", A64, iDA)
    iL = np.linalg.inv(Lm)
    C = np.linalg.cholesky(iL)                         # iL = C C^T
    s = 1.0 / math.sqrt(2.0)
    G = np.einsum("kdl,klm->kdm", iDA, C) * s          # (K, d, l)
    b = np.einsum("kd,kdl->kl", MU64, iDA)             # (K, l)
    h = np.einsum("kl,klm->km", b, C) * s              # (K, l)

    Gw = G.transpose(1, 0, 2).reshape(D_FEAT, GCOLS)   # col k*16+m
    Wx = (iD * MU64).T - 2.0 * np.einsum("kdm,km->kd", G, h).T
    Wxx = -0.5 * iD.T                                  # (d, K)

    det_L = np.linalg.slogdet(Lm)[1]
    log_det_sigma = det_L - np.sum(np.log(iD), axis=1)
    c1 = np.sum(iD * MU64 * MU64, axis=1)
    hsq = np.sum(h * h, axis=1)
    base = PI64 - 0.5 * (D_FEAT * math.log(2.0 * math.pi)
                         + log_det_sigma + c1) + hsq

    wcat = np.concatenate([Gw * SG, Wx], axis=1).astype(np.float32)  # (d, 2176)
    wq = np.ascontiguousarray(
        wcat.astype(FP8).reshape(J2, 2, 128, WCOLS).transpose(2, 0, 1, 3))

    wxxh = np.ascontiguousarray(
        Wxx.astype(np.float32).reshape(JJ, 128, K).transpose(1, 0, 2)
    ).astype(np.float16)

    s2 = np.zeros((128, NPAIR, 2, K), dtype=np.float32)
    p_idx = np.arange(128)
    for pair in range(NPAIR):
        for r in range(2):
            cb = 2 * pair + r
            s2[p_idx, pair, r, cb * 8 + p_idx // 16] = S2_VAL
    s2 = s2.astype(FP8)

    bs = np.ascontiguousarray(base.astype(np.float32).reshape(128, 1))
    return wq, wxxh, s2, bs


def kernel(x, PI, MU, A, D, _trace=False):
    from concourse.bass_utils import run_bass_kernel_spmd
    import ml_dtypes
    FP8 = ml_dtypes.float8_e4m3

    x = np.asarray(x, dtype=np.float32)
    wq, wxxh, s2, bs = _host_params(
        np.asarray(PI), np.asarray(MU), np.asarray(A), np.asarray(D))

    in_maps = []
    for c in range(N_CORES):
        xs = x[c * N_SHARD:(c + 1) * N_SHARD]          # (2048, 1024)
        x8t = xs.astype(FP8).T                         # (1024, 2048)
        xq = np.ascontiguousarray(
            x8t.reshape(J2, 2, 128, N_SHARD).transpose(2, 0, 1, 3))
        xsqt = (xs * xs).astype(np.float16).T          # (1024, 2048)
        xsqh = np.ascontiguousarray(
            xsqt.reshape(JJ, 128, N_SHARD).transpose(1, 0, 2))
        in_maps.append({
            "xq": xq,
            "xsqh": xsqh,
            "wq": wq,
            "wxxh": wxxh,
            "s2": s2,
            "bs": bs,
        })

    nc = _get_nc()
    res = run_bass_kernel_spmd(nc, in_maps, list(range(N_CORES)),
                               trace=_trace)
    _CACHE["last_results"] = res
    outT = np.concatenate([res.results[c]["outT"] for c in range(N_CORES)],
                          axis=1)                      # (128, 16384)
    return np.ascontiguousarray(outT.T).astype(np.float32)
